# revision 3
# baseline (speedup 1.0000x reference)
"""Bivariate Gaussian kernel (Nadaraya-Watson) on 8 TRN2 NeuronCores.

Math: for query m, result[m] = t[m] / (s[m] + EPS) where
  w[n,m] = exp(-c[m] * d2[n,m]),  c[m] = 1/(2*bw[m]^2)
  s[m] = sum_n w[n,m],  t[m] = sum_n w[n,m]*outputs[n]

Device algorithm (per core, M_loc=1024 queries = 2 m-blocks of 512):
  The PE computes S[n,m] = 128*log2(w[n,m]) + SBIAS as rank-11 matmuls
  (error-compensated bf16 hi/lo splits) into PSUM, 512-col blocks, with
  n-tile strips packed via row tile_position (bands j%4 -> rows 32b..32b+10).
  W = 2^z is then computed by BOTH non-PE compute engines concurrently:
   - ScalarE: ACTIVATE Exp with the free affine (scale=ln2/128,
     bias=-SBIAS*ln2/128), bf16 out.  ~125 G elem/s.
   - VectorE: a custom 8-stage DVE op (registered at import into
     concourse.dve_ops) that computes the bf16 BIT PATTERN of 2^z
     directly as an fp32 value -- magic-constant floor(z), quadratic
     minimax of 2^frac, +latched constant -- written through the
     uint16 write-port conversion (RNE, negatives clamp to 0).
     The uint16 tile is bitcast to bf16 for the reduce. ~113 G elem/s.
  n-tiles are split between the engines ~34/30 per m-block to balance.
  [s; t_hi; t_lo] accumulate over n by rank-3 reduce matmuls with
  stationary [ones, out_hi, out_lo], one PSUM bank, col tile_position
  per m-block.
Queries (M) are sharded across the 8 cores; each core sees all N points.
"""

import functools
import sys

import numpy as np

sys.path.insert(0, "/opt/trn_rl_repo")

EPS = 1e-7
N = 8192
M = 8192
NCORES = 8
MLOC = M // NCORES  # 1024
P = 128
NT = N // P  # 64 n-tiles
MBW = 512
MB = MLOC // MBW  # 2 m-blocks
K = 11  # compensated-split rank

LOG2E = 1.4426950408889634
MAGIC = 1.5 * 2.0**30
SBIAS = 64.0 + 126.0 * 128.0  # stream: S = 128*log2(w) + SBIAS
# minimax quad c2*F^2 + c1*F + c0 ~ 128*(2^((F+64)/128) - 1) on [-64.5,64.5]
C0U = 52.99109643311402
C1U = 0.9952810295418008
C2U = 0.002688034219766118

_ACT_NS = 1180.0  # fs=1024 ACTIVATE cadence
_D2_NS = 1262.0  # fs=1024 custom-DVE cadence
_D1_NS = 728.0  # fs=512


def _register_dve_op():
    import concourse.dve_ops as dvo
    from concourse.dve_spec import (
        Spec,
        Src0,
        C0,
        C1,
        C2,
        C3,
        _spill_c3_to_src1,
        lower,
    )
    from concourse.dve_uop import DveOpSpec

    name = "EXP2_BF16_PAT_ANT"
    if name in dvo._SUB_OPCODE_FOR_NAME:
        return next(op for op in dvo.OPS if op.name == name)

    t = Src0 + C0
    Kv = t - C0
    F = Src0 - Kv
    p = (C2 * F + C1) * F
    body = _spill_c3_to_src1((Kv + p) + C3)

    def ref(in0, in1, s0, s1, imm2):
        z = in0.astype(np.float32)
        tt = (z + np.float32(s0)).astype(np.float32)
        kk = (tt - np.float32(s0)).astype(np.float32)
        ff = (z - kk).astype(np.float32)
        pp = ((np.float32(imm2) * ff + np.float32(s1)) * ff).astype(np.float32)
        return (kk + pp).astype(np.float32) + in1.astype(np.float32).reshape(-1, 1)

    spec = Spec(body=body, reference=ref)
    row = max(dvo._SUB_OPCODE_FOR_NAME.values()) + 1
    assert row < 0x20
    shas = {}
    for ver in ("v3", "v4"):
        uops = lower(spec, ver=ver)
        s = DveOpSpec(name=name, opcode=row, uops=uops, rd1_en=True)
        shas[ver] = s.sha(ver)
    op = dvo.DveOp(name, spec, subdim=False, uops_sha=shas)
    dvo.OPS.append(op)
    dvo._SUB_OPCODE_FOR_NAME[name] = row
    dvo.CUSTOM_DVE_SPECS[name] = spec
    return op


def _schedule():
    """Per-m-block unit list: ("A",[j,j+1]) ACT units (17x2 tiles) and
    ("D",[...]) DVE units alternating 2/1 tiles (10x3 tiles)."""
    units = []
    tA = tD = 0.0
    j = 0
    remA, remD2, remD1 = 17, 10, 10
    d_big = True
    while j < NT:
        a_ok = remA > 0 and j + 1 < NT
        d_ok = (remD2 > 0 and j + 1 < NT) or remD1 > 0
        if a_ok and (tA <= tD or not d_ok):
            units.append(("A", [j, j + 1]))
            j += 2
            tA += _ACT_NS
            remA -= 1
        elif d_ok:
            if d_big and remD2 > 0 and j + 1 < NT:
                units.append(("D", [j, j + 1]))
                j += 2
                tD += _D2_NS
                remD2 -= 1
                d_big = False
            else:
                units.append(("D", [j]))
                j += 1
                tD += _D1_NS
                remD1 -= 1
                d_big = True
        else:
            units.append(("A", [j, j + 1]))
            j += 2
            remA -= 1
    assert j == NT
    return units


@functools.lru_cache(maxsize=1)
def _build():
    import concourse.tile as tile
    from concourse import bacc, mybir

    f32 = mybir.dt.float32
    bf16 = mybir.dt.bfloat16
    u16 = mybir.dt.uint16
    EXP = mybir.ActivationFunctionType.Exp

    op = _register_dve_op()
    units = _schedule()

    nc = bacc.Bacc("TRN2", target_bir_lowering=False, debug=False, num_devices=NCORES)
    # stat: strip for n-tile j at rows 32*(j%4)..+10, cols (j//4)*128..+127
    stat_d = nc.dram_tensor("stat", [P, (NT // 4) * P], bf16, kind="ExternalInput")
    # mov: every 32-row band holds the same 11 rows (PQRS hi/lo splits, scaled)
    mov_d = nc.dram_tensor("mov", [P, MLOC], bf16, kind="ExternalInput")
    rsb_d = nc.dram_tensor("rsb", [P, 4 * NT], bf16, kind="ExternalInput")
    res_d = nc.dram_tensor("res", [3 * MB, MBW], f32, kind="ExternalOutput")

    with tile.TileContext(nc) as tc:
        with (
            tc.tile_pool(name="const", bufs=1) as cpool,
            tc.tile_pool(name="wa", bufs=5) as wpa,
            tc.tile_pool(name="wd", bufs=5) as wpd,
            tc.tile_pool(name="ea", bufs=2, space="PSUM") as pact,
            tc.tile_pool(name="ed", bufs=1, space="PSUM") as pdve,
            tc.tile_pool(name="acc", bufs=1, space="PSUM") as apool,
        ):
            # warm-ups on never-written junk: PE pipeline + ACT table preload
            # + custom-DVE first-dispatch, all before input DMAs land.
            junk = cpool.tile([P, MBW], bf16, tag="junk")
            nc.gpsimd.memset(junk[0:1, 0:1], 0.0)
            # per-partition consts for ACT affine and the DVE latch
            bias_t = cpool.tile([P, 1], f32, tag="bias")
            scale_t = cpool.tile([P, 1], f32, tag="scale")
            c3t = cpool.tile([P, 1], f32, tag="c3t")
            ln2_128 = float(np.log(2.0) / 128.0)
            nc.gpsimd.memset(bias_t[:], -SBIAS * ln2_128)
            nc.gpsimd.memset(scale_t[:], ln2_128)
            nc.gpsimd.memset(c3t[:], C0U)

            ed_warm = pact.tile([P, 1024], f32, tag="ea", name="ewarm")
            for _ in range(2):
                nc.tensor.matmul(
                    ed_warm[:, 0:MBW], junk[:, 0:P], junk[:], start=True, stop=True
                )
            scr2 = cpool.tile([1, 8], f32, tag="scr2")
            nc.scalar.activation(scr2[:], junk[0:1, 0:8], EXP)
            scrv = cpool.tile([1, 8], u16, tag="scrv")
            nc.vector._custom_dve(
                op,
                out=scrv[:],
                in0=junk[0:1, 0:8],
                in1=c3t[0:1, :],
                s0=MAGIC,
                s1=C1U,
                imm2=C2U,
            )

            # inputs: early-need chunks on hardware queues (sync/tensor),
            # late-need on the gpsimd software queue.
            stat = cpool.tile([P, (NT // 4) * P], bf16)
            mov = cpool.tile([P, MLOC], bf16)
            rsb = cpool.tile([P, 4 * NT], bf16)
            nc.sync.dma_start(mov[:, 0:MBW], mov_d[:, 0:MBW])
            nc.sync.dma_start(stat[:, 0 : 2 * P], stat_d[:, 0 : 2 * P])
            nc.scalar.dma_start(stat[:, 2 * P : 6 * P], stat_d[:, 2 * P : 6 * P])
            nc.sync.dma_start(stat[:, 6 * P : 10 * P], stat_d[:, 6 * P : 10 * P])
            nc.scalar.dma_start(rsb[:], rsb_d[:])
            nc.gpsimd.dma_start(stat[:, 10 * P : 16 * P], stat_d[:, 10 * P : 16 * P])
            nc.gpsimd.dma_start(mov[:, MBW:MLOC], mov_d[:, MBW:MLOC])

            acc = apool.tile([35, MBW], f32)

            started = [False] * MB
            reduced = [0] * MB
            pending = []

            def evict(h):
                st = cpool.tile([3, MBW], f32, tag=f"st{h}", name=f"st{h}")
                nc.vector.tensor_copy(st[:], acc[32 * h : 32 * h + 3, :])
                nc.gpsimd.dma_start(res_d[3 * h : 3 * h + 3, :], st[:])

            def emit_reduce(w, h, tiles, is_dve):
                for idx, j in enumerate(tiles):
                    mv = w[:, idx * MBW : (idx + 1) * MBW]
                    if is_dve:
                        mv = mv.bitcast(bf16)
                    nc.tensor.matmul(
                        acc[32 * h : 32 * h + 3, :],
                        rsb[:, 4 * j : 4 * j + 3],
                        mv,
                        start=not started[h],
                        stop=reduced[h] == NT - 1,
                        tile_position=(0, 32 * h),
                    )
                    started[h] = True
                    reduced[h] += 1
                if reduced[h] == NT:
                    evict(h)

            for h in range(MB):
                for kind, tiles in units:
                    fs = len(tiles) * MBW
                    if kind == "A":
                        e = pact.tile([P, 1024], f32, tag="ea", name="ea")
                    else:
                        tag = "ed2" if len(tiles) == 2 else "ed1"
                        e = pdve.tile([P, fs], f32, tag=tag, name=tag)
                    for idx, j in enumerate(tiles):
                        b = j % 4
                        nc.tensor.matmul(
                            e[:, idx * MBW : (idx + 1) * MBW],
                            stat[32 * b : 32 * b + K, (j // 4) * P : (j // 4 + 1) * P],
                            mov[32 * b : 32 * b + K, h * MBW : (h + 1) * MBW],
                            start=True,
                            stop=True,
                            tile_position=(32 * b, 0),
                        )
                    if kind == "A":
                        w = wpa.tile([P, fs], bf16, tag="wa", name="wa")
                        nc.scalar.activation(
                            w[:], e[:, 0:fs], EXP, bias=bias_t[:], scale=scale_t[:]
                        )
                    else:
                        wtag = "wd2" if len(tiles) == 2 else "wd1"
                        w = wpd.tile([P, fs], u16, tag=wtag, name=wtag)
                        nc.vector._custom_dve(
                            op,
                            out=w[:],
                            in0=e[:, 0:fs],
                            in1=c3t[:],
                            s0=MAGIC,
                            s1=C1U,
                            imm2=C2U,
                        )
                    pending.append((w, h, tiles, kind == "D"))
                    if len(pending) > 3:
                        emit_reduce(*pending.pop(0))
            for args in pending:
                emit_reduce(*args)

    nc.compile()
    return nc


def _bf16_split(v):
    import ml_dtypes

    hi = v.astype(ml_dtypes.bfloat16)
    lo = (v - hi.astype(np.float64)).astype(ml_dtypes.bfloat16)
    return hi, lo


def _prepare(x, inputs, outputs, bandwidth):
    """Host-side O(N+M) prep of the factored operands (pattern-scaled)."""
    import ml_dtypes

    in0 = inputs[:, 0].astype(np.float64)
    in1 = inputs[:, 1].astype(np.float64)
    a2 = in0 * in0 + in1 * in1
    x0 = x[:, 0].astype(np.float64)
    x1 = x[:, 1].astype(np.float64)
    b2 = x0 * x0 + x1 * x1
    c = 1.0 / (2.0 * bandwidth.astype(np.float64) ** 2)
    A = 128.0 * LOG2E
    Pm = A * (-c * b2) + SBIAS
    Qm = A * (-c)
    Rm = A * (2.0 * c * x0)
    Sm = A * (2.0 * c * x1)

    ones = np.ones(N, np.float64)
    a2h, a2l = _bf16_split(a2)
    i0h, i0l = _bf16_split(in0)
    i1h, i1l = _bf16_split(in1)
    oneh, _ = _bf16_split(ones)
    Ph, Pl = _bf16_split(Pm)
    Qh, Ql = _bf16_split(Qm)
    Rh, Rl = _bf16_split(Rm)
    Sh, Sl = _bf16_split(Sm)

    stat_rows = np.stack(
        [oneh, oneh, a2h, a2h, a2l, i0h, i0h, i0l, i1h, i1h, i1l]
    )  # (K, N)
    mov_rows = np.stack([Ph, Pl, Qh, Ql, Qh, Rh, Rl, Rh, Sh, Sl, Sh])  # (K, M)

    stat = np.zeros((P, (NT // 4) * P), ml_dtypes.bfloat16)
    for j in range(NT):
        b = j % 4
        stat[32 * b : 32 * b + K, (j // 4) * P : (j // 4 + 1) * P] = stat_rows[
            :, j * P : (j + 1) * P
        ]
    mov = np.zeros((P, M), ml_dtypes.bfloat16)
    for r in range(4):
        mov[32 * r : 32 * r + K, :] = mov_rows

    oh, ol = _bf16_split(outputs.astype(np.float64))
    rsb = np.zeros((N, 4), ml_dtypes.bfloat16)
    rsb[:, 0] = 1.0
    rsb[:, 1] = oh
    rsb[:, 2] = ol
    rsb_sb = np.ascontiguousarray(
        rsb.reshape(NT, P, 4).transpose(1, 0, 2).reshape(P, 4 * NT)
    )
    return stat, mov, rsb_sb


def kernel(x, inputs, outputs, bandwidth):
    from concourse.bass_utils import run_bass_kernel_spmd

    x = np.asarray(x, np.float32)
    inputs = np.asarray(inputs, np.float32)
    outputs = np.asarray(outputs, np.float32)
    bandwidth = np.asarray(bandwidth, np.float32)

    stat, mov, rsb_sb = _prepare(x, inputs, outputs, bandwidth)

    nc = _build()
    in_maps = [
        {
            "stat": stat,
            "mov": np.ascontiguousarray(mov[:, c * MLOC : (c + 1) * MLOC]),
            "rsb": rsb_sb,
        }
        for c in range(NCORES)
    ]
    try:
        res = run_bass_kernel_spmd(nc, in_maps, list(range(NCORES)))
    except Exception:
        # transient NRT_EXEC_UNIT_UNRECOVERABLE after an interrupted prior
        # run; the device recovers after a short wait.
        import time

        time.sleep(20)
        res = run_bass_kernel_spmd(nc, in_maps, list(range(NCORES)))
    parts = []
    for c in range(NCORES):
        st = res.results[c]["res"]  # (6,512): [s,t_hi,t_lo] x {m-lo, m-hi}
        s = np.concatenate([st[0], st[3]])
        t = np.concatenate([st[1] + st[2], st[4] + st[5]])
        parts.append(t / (s + EPS))
    return np.concatenate(parts).astype(np.float32)


if __name__ == "__main__":
    rng = np.random.default_rng(0)
    x = rng.standard_normal((M, 2), np.float32)
    inputs = rng.standard_normal((N, 2), np.float32)
    outputs = rng.standard_normal(N, np.float32)
    bandwidth = (0.5 + rng.random(M)).astype(np.float32)
    got = kernel(x, inputs, outputs, bandwidth)
    print(got[:8])


# revision 6
# speedup vs baseline: 1.4357x; 1.4357x over previous
"""Bivariate Gaussian kernel (Nadaraya-Watson) on 8 TRN2 NeuronCores.

Math: for query m, result[m] = t[m] / (s[m] + EPS) where
  w[n,m] = exp(-c[m] * d2[n,m]),  c[m] = 1/(2*bw[m]^2)
  s[m] = sum_n w[n,m],  t[m] = sum_n w[n,m]*outputs[n]

Device algorithm (per core, M_loc=1024 queries = 2 m-blocks of 512):
  The PE computes S[n,m] = 128*log2(w[n,m]) + SBIAS as rank-11 matmuls
  (error-compensated bf16 hi/lo splits) into PSUM, 512-col blocks, with
  n-tile strips packed via row tile_position (bands j%4 -> rows 32b..32b+10).
  W = 2^z is then computed by BOTH non-PE compute engines concurrently:
   - ScalarE: ACTIVATE Exp with the free affine (scale=ln2/128,
     bias=-SBIAS*ln2/128), bf16 out.  ~125 G elem/s.
   - VectorE: a custom 8-stage DVE op (registered at import into
     concourse.dve_ops) that computes the bf16 BIT PATTERN of 2^z
     directly as an fp32 value -- magic-constant floor(z), quadratic
     minimax of 2^frac, +latched constant -- written through the
     uint16 write-port conversion (RNE, negatives clamp to 0).
     The uint16 tile is bitcast to bf16 for the reduce. ~113 G elem/s.
  n-tiles are split between the engines ~34/30 per m-block to balance.
  [s; t_hi; t_lo] accumulate over n by rank-3 reduce matmuls with
  stationary [ones, out_hi, out_lo], one PSUM bank, col tile_position
  per m-block.
Queries (M) are sharded across the 8 cores; each core sees all N points.
"""

import functools
import sys

import numpy as np

sys.path.insert(0, "/opt/trn_rl_repo")

EPS = 1e-7
N = 8192
M = 8192
NCORES = 8
MLOC = M // NCORES  # 1024
P = 128
NT = N // P  # 64 n-tiles
MBW = 512
MB = MLOC // MBW  # 2 m-blocks
K = 11  # compensated-split rank

LOG2E = 1.4426950408889634
MAGIC = 1.5 * 2.0**30
SBIAS = 64.0 + 126.0 * 128.0  # stream: S = 128*log2(w) + SBIAS
# minimax quad c2*F^2 + c1*F + c0 ~ 128*(2^((F+64)/128) - 1) on [-64.5,64.5]
C0U = 52.99109643311402
C1U = 0.9952810295418008
C2U = 0.002688034219766118

_ACT_NS = 1180.0  # fs=1024 ACTIVATE cadence
_D2_NS = 1262.0  # fs=1024 custom-DVE cadence
_D1_NS = 728.0  # fs=512


def _register_dve_op():
    import concourse.dve_ops as dvo
    from concourse.dve_spec import (
        Spec,
        Src0,
        C0,
        C1,
        C2,
        C3,
        _spill_c3_to_src1,
        lower,
    )
    from concourse.dve_uop import DveOpSpec

    name = "EXP2_BF16_PAT_ANT"
    if name in dvo._SUB_OPCODE_FOR_NAME:
        return next(op for op in dvo.OPS if op.name == name)

    t = Src0 + C0
    Kv = t - C0
    F = Src0 - Kv
    p = (C2 * F + C1) * F
    body = _spill_c3_to_src1((Kv + p) + C3)

    def ref(in0, in1, s0, s1, imm2):
        z = in0.astype(np.float32)
        tt = (z + np.float32(s0)).astype(np.float32)
        kk = (tt - np.float32(s0)).astype(np.float32)
        ff = (z - kk).astype(np.float32)
        pp = ((np.float32(imm2) * ff + np.float32(s1)) * ff).astype(np.float32)
        return (kk + pp).astype(np.float32) + in1.astype(np.float32).reshape(-1, 1)

    spec = Spec(body=body, reference=ref)
    row = max(dvo._SUB_OPCODE_FOR_NAME.values()) + 1
    assert row < 0x20
    shas = {}
    for ver in ("v3", "v4"):
        uops = lower(spec, ver=ver)
        s = DveOpSpec(name=name, opcode=row, uops=uops, rd1_en=True)
        shas[ver] = s.sha(ver)
    op = dvo.DveOp(name, spec, subdim=False, uops_sha=shas)
    dvo.OPS.append(op)
    dvo._SUB_OPCODE_FOR_NAME[name] = row
    dvo.CUSTOM_DVE_SPECS[name] = spec
    return op


def _schedule():
    """Per-m-block step list. Each step is 1-3 units (kind, [tiles]) whose
    E strips issue adjacently on the PE (consecutive j -> distinct row
    bands mod 4). ACT units: 17x2 tiles; DVE units alternate 2/1 tiles
    (10 of each). Steps pair one ACT unit with DVE unit(s), clock-balanced."""
    steps = []
    j = 0
    remA, remD = 17, 20
    d_big = True
    tA = tD = 0.0
    while remA or remD:
        step = []
        if remA:
            step.append(("A", 2))
            remA -= 1
            tA += _ACT_NS
        for _ in range(2):
            if remD and (tD <= tA or not remA):
                n = 2 if d_big else 1
                step.append(("D", n))
                remD -= 1
                tD += _D2_NS if d_big else _D1_NS
                d_big = not d_big
        out = []
        for k, n in step:
            out.append((k, list(range(j, j + n))))
            j += n
        steps.append(out)
    assert j == NT, j
    return steps


@functools.lru_cache(maxsize=1)
def _build():
    import concourse.tile as tile
    from concourse import bacc, mybir

    f32 = mybir.dt.float32
    bf16 = mybir.dt.bfloat16
    u16 = mybir.dt.uint16
    EXP = mybir.ActivationFunctionType.Exp

    op = _register_dve_op()
    units = _schedule()

    nc = bacc.Bacc("TRN2", target_bir_lowering=False, debug=False, num_devices=NCORES)
    # stat: strip for n-tile j at rows 32*(j%4)..+10, cols (j//4)*128..+127
    stat_d = nc.dram_tensor("stat", [P, (NT // 4) * P], bf16, kind="ExternalInput")
    # mov: every 32-row band holds the same 11 rows (PQRS hi/lo splits, scaled)
    mov_d = nc.dram_tensor("mov", [P, MLOC], bf16, kind="ExternalInput")
    rsb_d = nc.dram_tensor("rsb", [P, 4 * NT], bf16, kind="ExternalInput")
    res_d = nc.dram_tensor("res", [3 * MB, MBW], f32, kind="ExternalOutput")

    with tile.TileContext(nc) as tc:
        with (
            tc.tile_pool(name="const", bufs=1) as cpool,
            tc.tile_pool(name="wa", bufs=5) as wpa,
            tc.tile_pool(name="wd", bufs=5) as wpd,
            tc.tile_pool(name="ea", bufs=2, space="PSUM") as pact,
            tc.tile_pool(name="ed", bufs=1, space="PSUM") as pdve,
            tc.tile_pool(name="acc", bufs=1, space="PSUM") as apool,
        ):
            # warm-ups on never-written junk: PE pipeline + ACT table preload
            # + custom-DVE first-dispatch, all before input DMAs land.
            junk = cpool.tile([P, MBW], bf16, tag="junk")
            nc.gpsimd.memset(junk[0:1, 0:1], 0.0)
            # per-partition consts for ACT affine and the DVE latch
            bias_t = cpool.tile([P, 1], f32, tag="bias")
            scale_t = cpool.tile([P, 1], f32, tag="scale")
            c3t = cpool.tile([P, 1], f32, tag="c3t")
            ln2_128 = float(np.log(2.0) / 128.0)
            nc.gpsimd.memset(bias_t[:], -SBIAS * ln2_128)
            nc.gpsimd.memset(scale_t[:], ln2_128)
            nc.gpsimd.memset(c3t[:], C0U)

            ed_warm = pact.tile([P, 1024], f32, tag="ea", name="ewarm")
            for _ in range(2):
                nc.tensor.matmul(
                    ed_warm[:, 0:MBW], junk[:, 0:P], junk[:], start=True, stop=True
                )
            scr2 = cpool.tile([1, 8], f32, tag="scr2")
            nc.scalar.activation(scr2[:], junk[0:1, 0:8], EXP)
            scrv = cpool.tile([1, 8], u16, tag="scrv")
            nc.vector._custom_dve(
                op,
                out=scrv[:],
                in0=junk[0:1, 0:8],
                in1=c3t[0:1, :],
                s0=MAGIC,
                s1=C1U,
                imm2=C2U,
            )

            # inputs: early-need chunks on hardware queues (sync/tensor),
            # late-need on the gpsimd software queue.
            stat = cpool.tile([P, (NT // 4) * P], bf16)
            mov = cpool.tile([P, MLOC], bf16)
            rsb = cpool.tile([P, 4 * NT], bf16)
            nc.sync.dma_start(mov[:, 0:MBW], mov_d[:, 0:MBW])
            nc.sync.dma_start(stat[:, 0 : 2 * P], stat_d[:, 0 : 2 * P])
            nc.scalar.dma_start(stat[:, 2 * P : 6 * P], stat_d[:, 2 * P : 6 * P])
            nc.sync.dma_start(stat[:, 6 * P : 10 * P], stat_d[:, 6 * P : 10 * P])
            nc.scalar.dma_start(rsb[:], rsb_d[:])
            nc.gpsimd.dma_start(stat[:, 10 * P : 16 * P], stat_d[:, 10 * P : 16 * P])
            nc.gpsimd.dma_start(mov[:, MBW:MLOC], mov_d[:, MBW:MLOC])

            acc = apool.tile([35, MBW], f32)

            started = [False] * MB
            reduced = [0] * MB
            pending = []

            def evict(h):
                st = cpool.tile([3, MBW], f32, tag=f"st{h}", name=f"st{h}")
                nc.vector.tensor_copy(st[:], acc[32 * h : 32 * h + 3, :])
                nc.gpsimd.dma_start(res_d[3 * h : 3 * h + 3, :], st[:])

            def emit_reduce(w, h, tiles, is_dve):
                for idx, j in enumerate(tiles):
                    mv = w[:, idx * MBW : (idx + 1) * MBW]
                    if is_dve:
                        mv = mv.bitcast(bf16)
                    nc.tensor.matmul(
                        acc[32 * h : 32 * h + 3, :],
                        rsb[:, 4 * j : 4 * j + 3],
                        mv,
                        start=not started[h],
                        stop=reduced[h] == NT - 1,
                        tile_position=(0, 32 * h),
                    )
                    started[h] = True
                    reduced[h] += 1
                if reduced[h] == NT:
                    evict(h)

            def fill_e(h, kind, tiles):
                fs = len(tiles) * MBW
                if kind == "A":
                    e = pact.tile([P, 1024], f32, tag="ea", name="ea")
                else:
                    tag = "ed2" if len(tiles) == 2 else "ed1"
                    e = pdve.tile([P, fs], f32, tag=tag, name=tag)
                for idx, j in enumerate(tiles):
                    b = j % 4
                    nc.tensor.matmul(
                        e[:, idx * MBW : (idx + 1) * MBW],
                        stat[32 * b : 32 * b + K, (j // 4) * P : (j // 4 + 1) * P],
                        mov[32 * b : 32 * b + K, h * MBW : (h + 1) * MBW],
                        start=True,
                        stop=True,
                        tile_position=(32 * b, 0),
                    )
                return e

            def consume(h, kind, tiles, e):
                fs = len(tiles) * MBW
                if kind == "A":
                    w = wpa.tile([P, fs], bf16, tag="wa", name="wa")
                    nc.scalar.activation(
                        w[:], e[:, 0:fs], EXP, bias=bias_t[:], scale=scale_t[:]
                    )
                else:
                    wtag = "wd2" if len(tiles) == 2 else "wd1"
                    w = wpd.tile([P, fs], u16, tag=wtag, name=wtag)
                    nc.vector._custom_dve(
                        op,
                        out=w[:],
                        in0=e[:, 0:fs],
                        in1=c3t[:],
                        s0=MAGIC,
                        s1=C1U,
                        imm2=C2U,
                    )
                return w

            # macro-steps: each step's E strips (3-5, distinct bands) issue
            # adjacently and row-pack on the PE; reduces then go out in
            # back-to-back runs.
            for h in range(MB):
                for step in units:
                    es = [fill_e(h, k, t) for k, t in step]
                    for (k, t), e in zip(step, es):
                        pending.append((consume(h, k, t, e), h, t, k == "D"))
                    while len(pending) > 4:
                        emit_reduce(*pending.pop(0))
            for args in pending:
                emit_reduce(*args)

    nc.compile()
    return nc


def _bf16_split(v):
    import ml_dtypes

    hi = v.astype(ml_dtypes.bfloat16)
    lo = (v - hi.astype(np.float64)).astype(ml_dtypes.bfloat16)
    return hi, lo


def _prepare(x, inputs, outputs, bandwidth):
    """Host-side O(N+M) prep of the factored operands (pattern-scaled)."""
    import ml_dtypes

    in0 = inputs[:, 0].astype(np.float64)
    in1 = inputs[:, 1].astype(np.float64)
    a2 = in0 * in0 + in1 * in1
    x0 = x[:, 0].astype(np.float64)
    x1 = x[:, 1].astype(np.float64)
    b2 = x0 * x0 + x1 * x1
    c = 1.0 / (2.0 * bandwidth.astype(np.float64) ** 2)
    A = 128.0 * LOG2E
    Pm = A * (-c * b2) + SBIAS
    Qm = A * (-c)
    Rm = A * (2.0 * c * x0)
    Sm = A * (2.0 * c * x1)

    ones = np.ones(N, np.float64)
    a2h, a2l = _bf16_split(a2)
    i0h, i0l = _bf16_split(in0)
    i1h, i1l = _bf16_split(in1)
    oneh, _ = _bf16_split(ones)
    Ph, Pl = _bf16_split(Pm)
    Qh, Ql = _bf16_split(Qm)
    Rh, Rl = _bf16_split(Rm)
    Sh, Sl = _bf16_split(Sm)

    stat_rows = np.stack(
        [oneh, oneh, a2h, a2h, a2l, i0h, i0h, i0l, i1h, i1h, i1l]
    )  # (K, N)
    mov_rows = np.stack([Ph, Pl, Qh, Ql, Qh, Rh, Rl, Rh, Sh, Sl, Sh])  # (K, M)

    stat = np.zeros((P, (NT // 4) * P), ml_dtypes.bfloat16)
    for j in range(NT):
        b = j % 4
        stat[32 * b : 32 * b + K, (j // 4) * P : (j // 4 + 1) * P] = stat_rows[
            :, j * P : (j + 1) * P
        ]
    mov = np.zeros((P, M), ml_dtypes.bfloat16)
    for r in range(4):
        mov[32 * r : 32 * r + K, :] = mov_rows

    oh, ol = _bf16_split(outputs.astype(np.float64))
    rsb = np.zeros((N, 4), ml_dtypes.bfloat16)
    rsb[:, 0] = 1.0
    rsb[:, 1] = oh
    rsb[:, 2] = ol
    rsb_sb = np.ascontiguousarray(
        rsb.reshape(NT, P, 4).transpose(1, 0, 2).reshape(P, 4 * NT)
    )
    return stat, mov, rsb_sb


def kernel(x, inputs, outputs, bandwidth):
    from concourse.bass_utils import run_bass_kernel_spmd

    x = np.asarray(x, np.float32)
    inputs = np.asarray(inputs, np.float32)
    outputs = np.asarray(outputs, np.float32)
    bandwidth = np.asarray(bandwidth, np.float32)

    stat, mov, rsb_sb = _prepare(x, inputs, outputs, bandwidth)

    nc = _build()
    in_maps = [
        {
            "stat": stat,
            "mov": np.ascontiguousarray(mov[:, c * MLOC : (c + 1) * MLOC]),
            "rsb": rsb_sb,
        }
        for c in range(NCORES)
    ]
    try:
        res = run_bass_kernel_spmd(nc, in_maps, list(range(NCORES)))
    except Exception:
        # transient NRT_EXEC_UNIT_UNRECOVERABLE after an interrupted prior
        # run; the device recovers after a short wait.
        import time

        time.sleep(20)
        res = run_bass_kernel_spmd(nc, in_maps, list(range(NCORES)))
    parts = []
    for c in range(NCORES):
        st = res.results[c]["res"]  # (6,512): [s,t_hi,t_lo] x {m-lo, m-hi}
        s = np.concatenate([st[0], st[3]])
        t = np.concatenate([st[1] + st[2], st[4] + st[5]])
        parts.append(t / (s + EPS))
    return np.concatenate(parts).astype(np.float32)


if __name__ == "__main__":
    rng = np.random.default_rng(0)
    x = rng.standard_normal((M, 2), np.float32)
    inputs = rng.standard_normal((N, 2), np.float32)
    outputs = rng.standard_normal(N, np.float32)
    bandwidth = (0.5 + rng.random(M)).astype(np.float32)
    got = kernel(x, inputs, outputs, bandwidth)
    print(got[:8])
